# revision 1
# baseline (speedup 1.0000x reference)
"""Trainium2 Bass kernel for nn_Net_12266426597866 (GNN message passing).

Strategy (8 NeuronCores, SPMD):
  - Dense normalized adjacency, column(dst)-sharded: each core builds its
    2000x250 slice of A_w^T (summed edge weights) ON PE by accumulating
    one-hot outer products (one-hots built by iota-compare on DVE, bf16),
    which also handles parallel-edge accumulation for free. deg = row sums
    + AllGather; the two ChebConvs collapse algebraically to two 9-column
    matmuls against the A_w slice:
        h2 = X@M0 + t1@M1 + u@M2 + rs*beta + alpha,
        t1 = W@X, u = W@t1, W = -S A_w S  (S applied to rhs / out).
  - Temporal convs on PE with block-Toeplitz packed weights.
  - Sparse-softmax attention: per-edge a[src]/a[dst] fetched with one
    GPSIMD dma_gather (256B rows of a replicated table), exp + masked
    partial sums, AllGather for the global softmax denominator.
  - div_op matmul collapses to a 0/1 dedup'd adjacency B (min-clamped PE
    one-hot accumulation) times the 2000x8 matrix ea*diff*A[:2000].
  - Output: rows 0..5999 are exact copies of the input; device computes
    the 2000 x_new rows (250 per core).

All floating-point compute happens on device; the host only reorders /
partitions indices (sharding tables) and assembles the output.
"""

import os
import sys

sys.path.insert(0, "/opt/trn_rl_repo")

import numpy as np

import concourse.bass as bass
import concourse.bacc as bacc
import concourse.mybir as mybir
import concourse.tile as tile
from concourse.masks import make_identity

F32 = mybir.dt.float32
BF16 = mybir.dt.bfloat16
I32 = mybir.dt.int32
I16 = mybir.dt.int16
AX = mybir.AxisListType
OP = mybir.AluOpType
ACT = mybir.ActivationFunctionType

# problem sizes
N, E, T, F = 2000, 32000, 4, 2
H, DK = 8, 16
C = 8                      # cores
NP, KT = 125, 16           # node tiling: n = p*KT + k  (p partition, k tile)
DSL = N // C               # 250 nodes (d-slice) per core
MH = 2                     # m-halves of d-slice (125 each)
CP = 256                   # padded column width of AW^T / B^T slices
EC = E // C                # own edge shard per core
NIDX = 12288               # dma_gather indices: own 4096*2 + prefix 2048*2
GCOLS = NIDX // 128        # 96


def _ceil(a, b):
    return -(-a // b)


class _Packer:
    def __init__(self, dtype):
        self.cols = {}
        self.w = 0
        self.dtype = dtype

    def add(self, name, ncols):
        self.cols[name] = (self.w, self.w + ncols)
        self.w += ncols

    def alloc(self, rows=128, pad_to=8):
        w = _ceil(self.w, pad_to) * pad_to
        return np.zeros((rows, w), self.dtype), w


def _prep(inputs):
    """Host-side shard/table construction (index manipulation only)."""
    x = np.asarray(inputs["x_list"], np.float32)[0]          # (8000, 2)
    ei = np.asarray(inputs["edge_index"]).astype(np.int64)
    src, dst = ei[0], ei[1]
    ew = np.asarray(inputs["edge_attr"], np.float32)

    # ---- per-core grouped edge lists for the PE one-hot builds
    ach = bch = 1
    for c in range(C):
        lo = c * DSL
        sel = np.where((dst >= lo) & (dst < lo + DSL))[0]
        cnt = np.bincount(src[sel] % KT, minlength=KT)
        ach = max(ach, _ceil(int(cnt.max()), 128))
        selb = np.where((src >= lo) & (src < lo + DSL))[0]
        cntb = np.bincount(dst[selb] % KT, minlength=KT)
        bch = max(bch, _ceil(int(cntb.max()), 128))
    ACH, BCH = ach, bch
    AWW, BWW = KT * ACH, KT * BCH

    fp = _Packer(np.float32)
    fp.add("awsp", AWW); fp.add("awdl", AWW); fp.add("awew", AWW)
    fp.add("bdp", BWW); fp.add("bsl", BWW)
    fp.add("emask", 32); fp.add("ea", KT); fp.add("xkt", KT * T * F)
    fp.add("xdl", MH * T * F)

    sp_ = _Packer(np.float32)
    for nm, w in [("w0", 2), ("w1", 2), ("w02T", 1), ("w12T", 1), ("b1", 1),
                  ("w1w", 1), ("w2wT", 2), ("w1b", 1), ("b2", 1), ("w2b", 2),
                  ("tc4b", 1), ("qka", 4), ("qkb", 4), ("hm", 8),
                  ("tc1wB", 64), ("tc2wB", 48), ("tc3wB", 32), ("tc4wB", 1),
                  ("tc1b", 1), ("tc2b", 1), ("tc3b", 1), ("msel", 2)]:
        sp_.add(nm, w)

    sm, SW = sp_.alloc()
    cs = sp_.cols

    def put(name, rows, arr):
        c0, c1 = cs[name]
        sm[:rows, c0:c1] = np.asarray(arr, np.float32).reshape(rows, c1 - c0)

    put("w0", 64, inputs["conv1_w0"])
    put("w1", 64, inputs["conv1_w1"])
    put("w02T", 64, np.asarray(inputs["conv2_w0"], np.float32).T)
    put("w12T", 64, np.asarray(inputs["conv2_w1"], np.float32).T)
    put("b1", 64, inputs["conv1_b"])
    put("w1w", 64, inputs["wout1_w"])
    put("w2wT", 64, np.asarray(inputs["wout2_w"], np.float32).T)
    put("w1b", 64, inputs["wout1_b"])
    put("b2", 1, inputs["conv2_b"])
    put("w2b", 1, np.asarray(inputs["wout2_b"], np.float32).reshape(1, 2))
    put("tc4b", 1, inputs["tc4_b"])
    qw = np.asarray(inputs["q_w"], np.float32)[:, 0]
    qb = np.asarray(inputs["q_b"], np.float32)
    kw = np.asarray(inputs["k_w"], np.float32)[:, 0]
    kb = np.asarray(inputs["k_b"], np.float32)
    put("qka", 128, np.stack([qw, qw, qb, qb], 1))
    put("qkb", 128, np.stack([kw, kb, kw, kb], 1))
    hm = (np.arange(128)[:, None] // DK == np.arange(H)[None, :]).astype(np.float32)
    put("hm", 128, hm)
    tc1 = np.asarray(inputs["tc1_w"], np.float32)
    tc2 = np.asarray(inputs["tc2_w"], np.float32)
    tc3 = np.asarray(inputs["tc3_w"], np.float32)
    tc4 = np.asarray(inputs["tc4_w"], np.float32)
    w1B = np.zeros((4, 64), np.float32)
    for t in range(4):
        for kk in range(3):
            r = t + kk - 1
            if 0 <= r < 4:
                w1B[r, t * 16:(t + 1) * 16] = tc1[:, 0, 0, kk]
    w2B = np.zeros((64, 48), np.float32)
    for t2 in range(3):
        for kk in range(2):
            w2B[(t2 + kk) * 16:(t2 + kk + 1) * 16, t2 * 16:(t2 + 1) * 16] = \
                tc2[:, :, 0, kk].T
    w3B = np.zeros((48, 32), np.float32)
    for t3 in range(2):
        for kk in range(2):
            w3B[(t3 + kk) * 16:(t3 + kk + 1) * 16, t3 * 16:(t3 + 1) * 16] = \
                tc3[:, :, 0, kk].T
    w4B = np.zeros((32, 1), np.float32)
    for kk in range(2):
        w4B[kk * 16:(kk + 1) * 16, 0] = tc4[0, :, 0, kk]
    put("tc1wB", 4, w1B); put("tc2wB", 64, w2B)
    put("tc3wB", 48, w3B); put("tc4wB", 32, w4B)
    put("tc1b", 64, np.tile(np.asarray(inputs["tc1_b"], np.float32), T))
    put("tc2b", 48, np.tile(np.asarray(inputs["tc2_b"], np.float32), 3))
    put("tc3b", 32, np.tile(np.asarray(inputs["tc3_b"], np.float32), 2))
    msel = np.zeros((16, 2), np.float32)
    msel[np.arange(16), np.arange(16) // C] = 1.0
    put("msel", 16, msel)

    pidx = np.arange(NP)
    jj = (pidx[:, None] * KT + np.arange(KT)[None, :])       # j or n = p*16+k

    in_maps = []
    for c in range(C):
        ft, FW = fp.alloc(rows=128)
        fc = fp.cols

        def fput(name, arr2d):
            c0, c1 = fc[name]
            ft[: arr2d.shape[0], c0:c1] = arr2d

        lo = c * DSL
        sel = np.where((dst >= lo) & (dst < lo + DSL))[0]
        sp = np.full((KT, ACH * 128), -1.0, np.float32)
        dl = np.full((KT, ACH * 128), -1.0, np.float32)
        wv = np.zeros((KT, ACH * 128), np.float32)
        for k in range(KT):
            e = sel[src[sel] % KT == k]
            sp[k, :len(e)] = src[e] // KT
            dl[k, :len(e)] = dst[e] - lo
            wv[k, :len(e)] = ew[e]
        # layout [128 rows, (k, ch) cols]: row = edge-in-chunk
        fput("awsp", sp.reshape(KT * ACH, 128).T)
        fput("awdl", dl.reshape(KT * ACH, 128).T)
        fput("awew", wv.reshape(KT * ACH, 128).T)
        selb = np.where((src >= lo) & (src < lo + DSL))[0]
        dp = np.full((KT, BCH * 128), -1.0, np.float32)
        sl = np.full((KT, BCH * 128), -1.0, np.float32)
        for k in range(KT):
            e = selb[dst[selb] % KT == k]
            dp[k, :len(e)] = dst[e] // KT
            sl[k, :len(e)] = src[e] - lo
        fput("bdp", dp.reshape(KT * BCH, 128).T)
        fput("bsl", sl.reshape(KT * BCH, 128).T)

        em = np.zeros(32 * 128, np.float32)
        em[:EC] = 1.0
        fput("emask", em.reshape(32, 128).T)    # emask[p, cb], l = cb*128+p
        fput("ea", ew[jj])
        xk = np.zeros((NP, KT, T, F), np.float32)
        for t in range(T):
            xk[:, :, t, :] = x[t * N + jj]
        fput("xkt", xk.reshape(NP, KT * T * F))
        xd = np.zeros((NP, MH, T, F), np.float32)
        for m in range(MH):
            for t in range(T):
                xd[:, m, t, :] = x[t * N + c * DSL + m * NP + pidx]
        fput("xdl", xd.reshape(NP, MH * T * F))

        # ---- int32 tables: deg d-local run-gather offsets (row q = (m, r))
        it = np.zeros((16, 8), np.int32)
        for q in range(16):
            m, r = q // C, q % C
            it[q, 0] = r * N + c * DSL + m * NP
        # ---- dma_gather index list: slot i -> out[i%128, i//128, 0]
        gl = np.zeros(NIDX, np.int64)
        e0 = c * EC
        gl[0:EC] = src[e0:e0 + EC]               # own-as, l = i
        gl[4096:4096 + EC] = dst[e0:e0 + EC]     # own-ad, l = i - 4096
        for k in range(KT):                       # prefix j = p*16+k
            gl[(64 + k) * 128: (64 + k) * 128 + NP] = src[jj[:, k]]
            gl[(80 + k) * 128: (80 + k) * 128 + NP] = dst[jj[:, k]]
        g16t = np.zeros((16, NIDX // 16), np.int16)
        ii = np.arange(NIDX)
        g16t[ii % 16, ii // 16] = gl.astype(np.int16)
        g16t = np.tile(g16t, (8, 1))
        in_maps.append({"smalls": sm, "ftabs": ft, "itabs": it, "g16": g16t})

    widths = dict(ACH=ACH, BCH=BCH, SW=SW, FW=FW,
                  fcols=dict(fp.cols), scols=dict(sp_.cols))
    return in_maps, widths, x


def _split_multi_waits(nc):
    """Walrus codegen in this container accepts only one inline sync wait per
    instruction; hoist extras into standalone EventSemaphore waits."""
    for func in nc.m.functions:
        for bb in func.blocks:
            out = []
            for inst in bb.instructions:
                si = inst.sync_info
                waits = list(si.on_wait) if (si is not None and si.on_wait) else []
                if len(waits) > 1:
                    for w in waits[:-1]:
                        out.append(mybir.InstEventSemaphore(
                            name=nc.get_next_instruction_name(),
                            engine=inst.engine, ins=[], outs=[],
                            sync_info=mybir.SyncInfo(on_wait=[w], on_update=[])))
                    inst.sync_info = mybir.SyncInfo(on_wait=[waits[-1]],
                                                    on_update=list(si.on_update))
                out.append(inst)
            bb.instructions = out


def _build(w, split=True):
    """Construct the SPMD Bass program (identical across cores)."""
    nc = bacc.Bacc(None, num_devices=C)
    fc, sc = w["fcols"], w["scols"]
    ACH, BCH = w["ACH"], w["BCH"]

    smalls = nc.declare_dram_parameter("smalls", [128, w["SW"]], F32, isOutput=False)
    ftabs = nc.declare_dram_parameter("ftabs", [128, w["FW"]], F32, isOutput=False)
    itabs = nc.declare_dram_parameter("itabs", [16, 8], I32, isOutput=False)
    g16 = nc.declare_dram_parameter("g16", [128, NIDX // 16], I16, isOutput=False)
    xnew = nc.declare_dram_parameter("xnew", [DSL, F], F32, isOutput=True)

    deg_in = nc.dram_tensor("deg_in", [NP, KT], F32)
    deg_out = nc.dram_tensor("deg_out", [C, N], F32, addr_space="Shared")
    t1_in = nc.dram_tensor("t1_in", [DSL, 8], F32)
    t1_out = nc.dram_tensor("t1_out", [N, 8], F32, addr_space="Shared")
    a_in = nc.dram_tensor("a_in", [1, DSL], F32)
    a_out = nc.dram_tensor("a_out", [1, N], F32, addr_space="Shared")
    z_in = nc.dram_tensor("z_in", [1, H], F32)
    z_out = nc.dram_tensor("z_out", [C, H], F32, addr_space="Shared")
    arep = nc.dram_tensor("arep", [N, 64], F32)
    RG = [list(range(C))]

    IOA = bass.IndirectOffsetOnAxis

    with tile.TileContext(nc) as tc:
        with (
            tc.tile_pool(name="sb", bufs=1) as sb,
            tc.tile_pool(name="psb", bufs=2, space="PSUM") as psb,
            tc.tile_pool(name="ps9", bufs=2, space="PSUM") as ps9,
            tc.tile_pool(name="psg", bufs=2, space="PSUM") as psg,
            tc.tile_pool(name="pst", bufs=2, space="PSUM") as pst,
        ):
            def fsl(name):
                c0, c1 = fc[name]
                return ft_sb[:, c0:c1]

            def ssl(name, rows=64):
                c0, c1 = sc[name]
                return sm_sb[:rows, c0:c1]

            # ---------- stage inputs
            sm_sb = sb.tile([128, w["SW"]], F32, name="sm")
            ft_sb = sb.tile([128, w["FW"]], F32, name="ft")
            it_sb = sb.tile([16, 8], I32, name="it")
            g16_sb = sb.tile([128, NIDX // 16], I16, name="g16s")
            nc.sync.dma_start(sm_sb[:], smalls[:])
            nc.sync.dma_start(ft_sb[:], ftabs[:])
            nc.sync.dma_start(it_sb[:], itabs[:])
            nc.sync.dma_start(g16_sb[:], g16[:])
            id_sb = sb.tile([128, 128], F32, name="idm")
            make_identity(nc, id_sb[:])
            ones = sb.tile([128, 1], F32, name="ones")
            nc.gpsimd.memset(ones[:], 1.0)
            ones_mat = sb.tile([128, 128], F32, name="onesm")
            nc.gpsimd.memset(ones_mat[:], 1.0)
            iot = sb.tile([128, CP], F32, name="iot")
            nc.gpsimd.iota(iot[:], pattern=[[1, CP]], channel_multiplier=0,
                           allow_small_or_imprecise_dtypes=True)

            # ---------- build AW^T (bf16) by one-hot outer-product accumulation
            nch_a = KT * ACH
            os_a = sb.tile([128, nch_a, NP], BF16, name="osa")
            nc.vector.tensor_tensor(
                out=os_a[:],
                in0=iot[:, 0:NP].unsqueeze(1).to_broadcast([128, nch_a, NP]),
                in1=fsl("awsp").unsqueeze(2).to_broadcast([128, nch_a, NP]),
                op=OP.is_equal)
            nc.vector.tensor_tensor(
                out=os_a[:], in0=os_a[:],
                in1=fsl("awew").unsqueeze(2).to_broadcast([128, nch_a, NP]),
                op=OP.mult)
            od_a = sb.tile([128, nch_a, CP], BF16, name="oda")
            nc.vector.tensor_tensor(
                out=od_a[:],
                in0=iot[:].unsqueeze(1).to_broadcast([128, nch_a, CP]),
                in1=fsl("awdl").unsqueeze(2).to_broadcast([128, nch_a, CP]),
                op=OP.is_equal)

            aw_sb = sb.tile([NP, KT, CP], BF16, name="aw")
            for k in range(KT):
                ps = psb.tile([NP, CP], F32, tag="bld")
                for ch in range(ACH):
                    nc.tensor.matmul(out=ps[:], lhsT=os_a[:, k * ACH + ch, :],
                                     rhs=od_a[:, k * ACH + ch, :],
                                     start=(ch == 0), stop=(ch == ACH - 1))
                nc.vector.tensor_copy(aw_sb[:, k, :], ps[:])

            # ---------- build B^T (bf16, min-clamped for dedup)
            nch_b = KT * BCH
            os_b = sb.tile([128, nch_b, NP], BF16, name="osb")
            nc.vector.tensor_tensor(
                out=os_b[:],
                in0=iot[:, 0:NP].unsqueeze(1).to_broadcast([128, nch_b, NP]),
                in1=fsl("bdp").unsqueeze(2).to_broadcast([128, nch_b, NP]),
                op=OP.is_equal)
            od_b = sb.tile([128, nch_b, CP], BF16, name="odb")
            nc.vector.tensor_tensor(
                out=od_b[:],
                in0=iot[:].unsqueeze(1).to_broadcast([128, nch_b, CP]),
                in1=fsl("bsl").unsqueeze(2).to_broadcast([128, nch_b, CP]),
                op=OP.is_equal)
            b_sb = sb.tile([NP, KT, CP], BF16, name="bsb")
            for k in range(KT):
                ps = psb.tile([NP, CP], F32, tag="bld")
                for ch in range(BCH):
                    nc.tensor.matmul(out=ps[:], lhsT=os_b[:, k * BCH + ch, :],
                                     rhs=od_b[:, k * BCH + ch, :],
                                     start=(ch == 0), stop=(ch == BCH - 1))
                nc.vector.tensor_scalar(out=b_sb[:, k, :], in0=ps[:],
                                        scalar1=1.0, scalar2=None, op0=OP.min)

            # ---------- deg -> AG#1 -> inv-sqrt scalings
            degp = sb.tile([NP, KT], F32, name="degp")
            nc.vector.reduce_sum(out=degp[:], in_=aw_sb[:], axis=AX.X)
            nc.sync.dma_start(deg_in[:], degp[:])
            nc.gpsimd.collective_compute(
                "AllGather", OP.bypass, replica_groups=RG,
                ins=[deg_in[:]], outs=[deg_out[:]])
            degs = sb.tile([NP, C, KT], F32, name="degs")
            nc.sync.dma_start(
                degs[:], deg_out[:].rearrange("r (p k) -> p r k", p=NP))
            degkt = sb.tile([NP, KT], F32, name="degkt")
            nc.vector.reduce_sum(
                out=degkt[:], in_=degs[:].rearrange("p r k -> p k r"), axis=AX.X)

            sq = sb.tile([NP, KT], F32, name="sq")
            nc.scalar.sqrt(sq[:], degkt[:])
            rc = sb.tile([NP, KT], F32, name="rc")
            nc.vector.reciprocal(rc[:], sq[:])
            msk = sb.tile([NP, KT], F32, name="msk")
            nc.vector.tensor_scalar(out=msk[:], in0=degkt[:], scalar1=0.0,
                                    scalar2=None, op0=OP.is_gt)
            is_kt = sb.tile([NP, KT], F32, name="iskt")
            nc.vector.tensor_tensor(out=is_kt[:], in0=rc[:], in1=msk[:], op=OP.mult)
            is2 = sb.tile([NP, KT], F32, name="is2")
            nc.vector.scalar_tensor_tensor(out=is2[:], in0=is_kt[:], scalar=-1.0,
                                           in1=is_kt[:], op0=OP.mult, op1=OP.mult)

            # ---------- d-local -inv-sqrt via run-gather of deg_out
            dd8 = sb.tile([16, NP], F32, name="dd8")
            if os.environ.get("K_NO_DD8"):
                nc.gpsimd.memset(dd8[:], 1.0)
            else:
                nc.gpsimd.indirect_dma_start(
                    out=dd8[:], out_offset=None,
                    in_=deg_out[:], in_offset=IOA(ap=it_sb[:, 0:1], axis=1))
            ddp = pst.tile([2, NP], F32, tag="t")
            nc.tensor.matmul(out=ddp[:], lhsT=ssl("msel", 16), rhs=dd8[:],
                             start=True, stop=True)
            ddsb = sb.tile([2, NP], F32, name="ddsb")
            nc.vector.tensor_copy(ddsb[:], ddp[:])
            ddtp = pst.tile([NP, 2], F32, tag="t")
            nc.tensor.transpose(out=ddtp[:], in_=ddsb[:],
                                identity=id_sb[:2, :2])
            ddl = sb.tile([NP, MH], F32, name="ddl")
            nc.vector.tensor_copy(ddl[:], ddtp[:])
            sqd = sb.tile([NP, MH], F32, name="sqd")
            nc.scalar.sqrt(sqd[:], ddl[:])
            rcd = sb.tile([NP, MH], F32, name="rcd")
            nc.vector.reciprocal(rcd[:], sqd[:])
            mskd = sb.tile([NP, MH], F32, name="mskd")
            nc.vector.tensor_scalar(out=mskd[:], in0=ddl[:], scalar1=0.0,
                                    scalar2=None, op0=OP.is_gt)
            nisdl = sb.tile([NP, MH], F32, name="nisdl")
            nc.vector.scalar_tensor_tensor(out=nisdl[:], in0=rcd[:], scalar=-1.0,
                                           in1=mskd[:], op0=OP.mult, op1=OP.mult)

            # ---------- t1 = A_w @ (S x)   (9th col: S*ones for row sums)
            xkt = fsl("xkt")[:NP, :].rearrange("p (k c) -> p k c", k=KT)
            rhs1 = sb.tile([NP, KT, 9], BF16, name="rhs1")
            nc.vector.tensor_tensor(
                out=rhs1[:, :, 0:8], in0=xkt,
                in1=is_kt[:].unsqueeze(2).to_broadcast([NP, KT, 8]), op=OP.mult)
            nc.vector.tensor_copy(rhs1[:, :, 8], is_kt[:])

            ta_sb = sb.tile([NP, MH, 9], F32, name="ta")
            for m in range(MH):
                tp = ps9.tile([NP, 9], F32, tag="mm9")
                for k in range(KT):
                    nc.tensor.matmul(
                        out=tp[:], lhsT=aw_sb[:, k, m * NP:(m + 1) * NP],
                        rhs=rhs1[:, k, :], start=(k == 0), stop=(k == KT - 1))
                nc.vector.tensor_copy(ta_sb[:, m, :], tp[:])

            nc.sync.dma_start(
                t1_in[:].rearrange("(m p) c -> p m c", m=MH),
                ta_sb[:, :, 0:8])
            nc.gpsimd.collective_compute(
                "AllGather", OP.bypass, replica_groups=RG,
                ins=[t1_in[:]], outs=[t1_out[:]])
            t1f = sb.tile([NP, KT, 8], F32, name="t1f")
            nc.sync.dma_start(
                t1f[:], t1_out[:].rearrange("(p k) c -> p k c", p=NP))

            # ---------- u = A_w @ ((-S^2) ta_full)
            rhsu = sb.tile([NP, KT, 8], BF16, name="rhsu")
            nc.vector.tensor_tensor(
                out=rhsu[:], in0=t1f[:],
                in1=is2[:].unsqueeze(2).to_broadcast([NP, KT, 8]), op=OP.mult)
            ua_sb = sb.tile([NP, MH, 8], F32, name="ua")
            for m in range(MH):
                up = ps9.tile([NP, 8], F32, tag="mm9")
                for k in range(KT):
                    nc.tensor.matmul(
                        out=up[:], lhsT=aw_sb[:, k, m * NP:(m + 1) * NP],
                        rhs=rhsu[:, k, :], start=(k == 0), stop=(k == KT - 1))
                nc.vector.tensor_copy(ua_sb[:, m, :], up[:])

            # ---------- combined cheb weight row:
            # [M0_0, M0_1, M1_0, M1_1, M2_0, M2_1, alpha, beta]
            w02b2 = ssl("w02T").to_broadcast([64, 2])
            w12b2 = ssl("w12T").to_broadcast([64, 2])
            p8 = sb.tile([65, 8], F32, name="p8")
            nc.gpsimd.memset(p8[:], 0.0)
            scr2 = sb.tile([64, 2], F32, name="scr2")
            nc.vector.tensor_tensor(out=p8[0:64, 0:2], in0=ssl("w0"), in1=w02b2,
                                    op=OP.mult)
            nc.vector.tensor_tensor(out=p8[0:64, 2:4], in0=ssl("w1"), in1=w02b2,
                                    op=OP.mult)
            nc.vector.tensor_tensor(out=scr2[:], in0=ssl("w0"), in1=w12b2,
                                    op=OP.mult)
            nc.vector.tensor_tensor(out=p8[0:64, 2:4], in0=p8[0:64, 2:4],
                                    in1=scr2[:], op=OP.add)
            nc.vector.tensor_tensor(out=p8[0:64, 4:6], in0=ssl("w1"), in1=w12b2,
                                    op=OP.mult)
            nc.vector.tensor_tensor(out=p8[0:64, 6:7], in0=ssl("b1"),
                                    in1=ssl("w02T"), op=OP.mult)
            nc.vector.tensor_tensor(out=p8[0:64, 7:8], in0=ssl("b1"),
                                    in1=ssl("w12T"), op=OP.mult)
            nc.vector.tensor_copy(p8[64:65, 6:7], ssl("b2", 1))
            mrp = pst.tile([128, 8], F32, tag="t")
            nc.tensor.matmul(out=mrp[:], lhsT=ones_mat[0:65, :], rhs=p8[:],
                             start=True, stop=True)
            mb = sb.tile([128, 8], F32, name="mb")
            nc.vector.tensor_copy(mb[:], mrp[:])

            # ---------- h2 = Xterm + (-is_d) * Q
            xdl = fsl("xdl")[:NP, :].rearrange("p (m t f) -> p m t f", m=MH, t=T)
            xterm = sb.tile([NP, MH, T], F32, name="xterm")
            nc.vector.tensor_scalar(
                out=xterm[:], in0=xdl[:, :, :, 0],
                scalar1=mb[:NP, 0:1], scalar2=None, op0=OP.mult)
            nc.vector.scalar_tensor_tensor(
                out=xterm[:], in0=xdl[:, :, :, 1],
                scalar=mb[:NP, 1:2],
                in1=xterm[:], op0=OP.mult, op1=OP.add)
            nc.vector.tensor_scalar(
                out=xterm[:], in0=xterm[:],
                scalar1=mb[:NP, 6:7], scalar2=None,
                op0=OP.add)

            qq = sb.tile([NP, MH, T], F32, name="qq")
            nc.vector.tensor_scalar(
                out=qq[:], in0=ta_sb[:, :, 0:8:2],
                scalar1=mb[:NP, 2:3], scalar2=None,
                op0=OP.mult)
            nc.vector.scalar_tensor_tensor(
                out=qq[:], in0=ta_sb[:, :, 1:9:2],
                scalar=mb[:NP, 3:4],
                in1=qq[:], op0=OP.mult, op1=OP.add)
            nc.vector.scalar_tensor_tensor(
                out=qq[:], in0=ua_sb[:, :, 0:8:2],
                scalar=mb[:NP, 4:5],
                in1=qq[:], op0=OP.mult, op1=OP.add)
            nc.vector.scalar_tensor_tensor(
                out=qq[:], in0=ua_sb[:, :, 1:8:2],
                scalar=mb[:NP, 5:6],
                in1=qq[:], op0=OP.mult, op1=OP.add)
            nc.vector.scalar_tensor_tensor(
                out=qq[:], in0=ta_sb[:, :, 8:9].to_broadcast([NP, MH, T]),
                scalar=mb[:NP, 7:8],
                in1=qq[:], op0=OP.mult, op1=OP.add)

            h2 = sb.tile([NP, MH, T], F32, name="h2")
            for m in range(MH):
                nc.vector.scalar_tensor_tensor(
                    out=h2[:, m, :], in0=qq[:, m, :],
                    scalar=nisdl[:, m:m + 1],
                    in1=xterm[:, m, :], op0=OP.mult, op1=OP.add)

            # ---------- temporal convs ((tau, ch) on partitions, nodes free)
            hc4 = sb.tile([T, MH * NP], F32, name="hc4")
            for m in range(MH):
                tps = pst.tile([T, NP], F32, tag="t")
                nc.tensor.transpose(out=tps[:], in_=h2[:, m, :],
                                    identity=id_sb[:NP, :NP])
                nc.vector.tensor_copy(hc4[:, m * NP:(m + 1) * NP], tps[:])

            s1 = sb.tile([64, MH * NP], F32, name="s1t")
            cp1 = psb.tile([64, MH * NP], F32, tag="bld")
            nc.tensor.matmul(out=cp1[:], lhsT=ssl("tc1wB", 4),
                             rhs=hc4[:], start=True, stop=True)
            nc.vector.tensor_scalar(out=s1[:], in0=cp1[:],
                                    scalar1=ssl("tc1b"), scalar2=None,
                                    op0=OP.add)
            s2t = sb.tile([48, MH * NP], F32, name="s2t")
            cp2 = psb.tile([48, MH * NP], F32, tag="bld")
            nc.tensor.matmul(out=cp2[:], lhsT=ssl("tc2wB"),
                             rhs=s1[:], start=True, stop=True)
            nc.vector.tensor_scalar(out=s2t[:], in0=cp2[:],
                                    scalar1=ssl("tc2b", 48), scalar2=None,
                                    op0=OP.add)
            s3t = sb.tile([32, MH * NP], F32, name="s3t")
            cp3 = psb.tile([32, MH * NP], F32, tag="bld")
            nc.tensor.matmul(out=cp3[:], lhsT=ssl("tc3wB", 48),
                             rhs=s2t[:], start=True, stop=True)
            nc.vector.tensor_scalar(out=s3t[:], in0=cp3[:],
                                    scalar1=ssl("tc3b", 32), scalar2=None,
                                    op0=OP.add)
            ap4 = psb.tile([1, MH * NP], F32, tag="bld")
            nc.tensor.matmul(out=ap4[:], lhsT=ssl("tc4wB", 32),
                             rhs=s3t[:], start=True, stop=True)
            a_sb = sb.tile([1, MH * NP], F32, name="asb")
            nc.vector.tensor_scalar(out=a_sb[:], in0=ap4[:],
                                    scalar1=ssl("tc4b", 1), scalar2=None,
                                    op0=OP.add)
            nc.sync.dma_start(a_in[:], a_sb[:])
            nc.gpsimd.collective_compute(
                "AllGather", OP.bypass, replica_groups=RG,
                ins=[a_in[:]], outs=[a_out[:]])

            # ---------- replicated a table (2048, 64) + one dma_gather
            akt = sb.tile([NP, KT], F32, name="akt")
            nc.sync.dma_start(akt[:],
                              a_out[:].rearrange("o (p k) -> p (o k)", p=NP))
            arr = sb.tile([NP, KT, 64], F32, name="arr")
            nc.vector.tensor_copy(
                arr[:], akt[:].unsqueeze(2).to_broadcast([NP, KT, 64]))
            nc.sync.dma_start(
                arep[:].rearrange("(p k) c -> p k c", p=NP), arr[:])
            gat = sb.tile([128, GCOLS, 64], F32, name="gat")
            if os.environ.get("K_NO_GATHER"):
                nc.gpsimd.memset(gat[:], 0.001)
            else:
                from concourse.tile_rust import add_dep_helper
                prev = None
                for g in range(NIDX // 1024):
                    bi = nc.gpsimd.dma_gather(
                        out_ap=gat[:, g * 8:(g + 1) * 8, :], in_ap=arep[:],
                        idxs_ap=g16_sb[:, g * 64:(g + 1) * 64],
                        num_idxs=1024, num_idxs_reg=1024, elem_size=64)
                    if prev is not None:
                        add_dep_helper(prev.ins, bi.ins, sync=True,
                                       reason="swdge ring reclaim")
                    prev = bi

            # ---------- attention coefficients [kappa|s2|s3|s4] per head
            qp = sb.tile([128, 4], F32, name="qp")
            nc.vector.tensor_tensor(out=qp[:], in0=ssl("qka", 128),
                                    in1=ssl("qkb", 128), op=OP.mult)
            qh = sb.tile([128, 4, H], F32, name="qh")
            nc.vector.tensor_tensor(
                out=qh[:], in0=qp[:].unsqueeze(2).broadcast_to([128, 4, H]),
                in1=ssl("hm", 128).unsqueeze(1).broadcast_to([128, 4, H]),
                op=OP.mult)
            cfp = pst.tile([128, 4 * H], F32, tag="t")
            nc.tensor.matmul(out=cfp[:], lhsT=ones_mat[:], rhs=qh[:],
                             start=True, stop=True)
            cb = sb.tile([128, 4 * H], F32, name="cb")
            nc.scalar.mul(cb[:], cfp[:], 0.25)            # /sqrt(DK)=4

            def heads_own(row):
                return (cb[:, row * H:(row + 1) * H].unsqueeze(2)
                        .broadcast_to([128, H, 32]))

            def heads_pre(row):
                return (cb[:NP, row * H:(row + 1) * H].unsqueeze(1)
                        .broadcast_to([NP, KT, H]))

            # ---------- own-shard softmax partials
            aso = gat[:, 0:32, 0]
            ado = gat[:, 32:64, 0]
            asad = sb.tile([128, 32], F32, name="asad")
            nc.vector.tensor_tensor(out=asad[:], in0=aso, in1=ado, op=OP.mult)
            pown = sb.tile([128, H, 32], F32, name="pown")
            nc.vector.tensor_tensor(
                out=pown[:], in0=asad[:].unsqueeze(1).to_broadcast([128, H, 32]),
                in1=heads_own(0), op=OP.mult)
            tb = sb.tile([128, H, 32], F32, name="tb")
            nc.vector.tensor_tensor(
                out=tb[:], in0=aso.unsqueeze(1).to_broadcast([128, H, 32]),
                in1=heads_own(1), op=OP.mult)
            nc.vector.tensor_tensor(out=pown[:], in0=pown[:], in1=tb[:], op=OP.add)
            nc.vector.tensor_tensor(
                out=tb[:], in0=ado.unsqueeze(1).to_broadcast([128, H, 32]),
                in1=heads_own(2), op=OP.mult)
            nc.vector.tensor_tensor(out=pown[:], in0=pown[:], in1=tb[:], op=OP.add)
            nc.vector.tensor_tensor(
                out=pown[:], in0=pown[:],
                in1=heads_own(3), op=OP.add)
            eo = sb.tile([128, H, 32], F32, name="eo")
            nc.scalar.activation(eo[:], pown[:], ACT.Exp)
            nc.vector.tensor_tensor(
                out=eo[:], in0=eo[:],
                in1=fsl("emask").unsqueeze(1).to_broadcast([128, H, 32]),
                op=OP.mult)
            er = sb.tile([128, H], F32, name="er")
            nc.vector.reduce_sum(out=er[:], in_=eo[:], axis=AX.X)
            zp = pst.tile([1, H], F32, tag="t")
            nc.tensor.matmul(out=zp[:], lhsT=ones[:], rhs=er[:],
                             start=True, stop=True)
            zsb = sb.tile([1, H], F32, name="zsb")
            nc.vector.tensor_copy(zsb[:], zp[:])
            nc.sync.dma_start(z_in[:], zsb[:])
            nc.gpsimd.collective_compute(
                "AllGather", OP.bypass, replica_groups=RG,
                ins=[z_in[:]], outs=[z_out[:]])
            z8 = sb.tile([C, H], F32, name="z8")
            nc.sync.dma_start(z8[:], z_out[:])
            ztp = pst.tile([128, H], F32, tag="t")
            nc.tensor.matmul(out=ztp[:], lhsT=ones_mat[0:C, :], rhs=z8[:],
                             start=True, stop=True)
            zf = sb.tile([128, H], F32, name="zf")
            nc.vector.tensor_copy(zf[:], ztp[:])
            rzb = sb.tile([128, H], F32, name="rzb")
            nc.vector.reciprocal(rzb[:], zf[:])

            # ---------- prefix edges -> gxe rows (j = p*16+k)
            asp = gat[:NP, 64:80, 0]
            adp = gat[:NP, 80:96, 0]
            asadp = sb.tile([NP, KT], F32, name="asadp")
            nc.vector.tensor_tensor(out=asadp[:], in0=asp, in1=adp, op=OP.mult)
            ppre = sb.tile([NP, KT, H], F32, name="ppre")
            nc.vector.tensor_tensor(
                out=ppre[:], in0=asadp[:].unsqueeze(2).to_broadcast([NP, KT, H]),
                in1=heads_pre(0), op=OP.mult)
            tbp = sb.tile([NP, KT, H], F32, name="tbp")
            nc.vector.tensor_tensor(
                out=tbp[:], in0=asp.unsqueeze(2).to_broadcast([NP, KT, H]),
                in1=heads_pre(1), op=OP.mult)
            nc.vector.tensor_tensor(out=ppre[:], in0=ppre[:], in1=tbp[:], op=OP.add)
            nc.vector.tensor_tensor(
                out=tbp[:], in0=adp.unsqueeze(2).to_broadcast([NP, KT, H]),
                in1=heads_pre(2), op=OP.mult)
            nc.vector.tensor_tensor(out=ppre[:], in0=ppre[:], in1=tbp[:], op=OP.add)
            nc.vector.tensor_tensor(
                out=ppre[:], in0=ppre[:],
                in1=heads_pre(3), op=OP.add)
            epre = sb.tile([NP, KT, H], F32, name="epre")
            nc.scalar.activation(epre[:], ppre[:], ACT.Exp)
            dif = sb.tile([NP, KT], F32, name="dif")
            nc.vector.tensor_sub(dif[:], asp, adp)
            wpre = sb.tile([NP, KT], F32, name="wpre")
            nc.vector.tensor_tensor(out=wpre[:], in0=dif[:],
                                    in1=fsl("ea")[:NP, :], op=OP.mult)
            gxe = sb.tile([NP, KT, H], F32, name="gxe")
            nc.vector.tensor_tensor(
                out=gxe[:], in0=epre[:],
                in1=wpre[:].unsqueeze(2).to_broadcast([NP, KT, H]), op=OP.mult)
            gxb = sb.tile([NP, KT, H], BF16, name="gxb")
            nc.vector.tensor_tensor(
                out=gxb[:], in0=gxe[:],
                in1=rzb[:NP, :].unsqueeze(1).broadcast_to([NP, KT, H]),
                op=OP.mult)

            # ---------- ggx = B @ gxe ; x_new
            mh_s = sb.tile([NP, MH], F32, name="mhs")
            for m in range(MH):
                gp_ = psg.tile([NP, H], F32, tag="g")
                for k in range(KT):
                    nc.tensor.matmul(
                        out=gp_[:], lhsT=b_sb[:, k, m * NP:(m + 1) * NP],
                        rhs=gxb[:, k, :], start=(k == 0), stop=(k == KT - 1))
                nc.vector.reduce_sum(out=mh_s[:, m:m + 1], in_=gp_[:], axis=AX.X)

            vp = pst.tile([1, 2], F32, tag="t")
            nc.tensor.matmul(out=vp[:], lhsT=ssl("w1w"), rhs=ssl("w2wT"),
                             start=True, stop=True)
            cstp = pst.tile([1, 2], F32, tag="t")
            nc.tensor.matmul(out=cstp[:], lhsT=ssl("w1b"), rhs=ssl("w2wT"),
                             start=True, stop=True)
            vc = sb.tile([1, 4], F32, name="vc")
            nc.scalar.mul(vc[0:1, 0:2], vp[:], 1.0 / H)
            nc.vector.tensor_tensor(out=vc[0:1, 2:4], in0=cstp[:],
                                    in1=ssl("w2b", 1), op=OP.add)
            vcp = pst.tile([128, 4], F32, tag="t")
            nc.tensor.matmul(out=vcp[:], lhsT=ones_mat[0:1, :], rhs=vc[:],
                             start=True, stop=True)
            vcb = sb.tile([128, 4], F32, name="vcb")
            nc.vector.tensor_copy(vcb[:], vcp[:])

            xn = sb.tile([NP, MH, F], F32, name="xn")
            for m in range(MH):
                nc.vector.tensor_scalar(
                    out=xn[:, m, :],
                    in0=vcb[:NP, 0:2],
                    scalar1=mh_s[:, m:m + 1], scalar2=None, op0=OP.mult)
                nc.vector.tensor_tensor(
                    out=xn[:, m, :], in0=xn[:, m, :],
                    in1=vcb[:NP, 2:4], op=OP.add)
                nc.vector.tensor_tensor(
                    out=xn[:, m, :], in0=xn[:, m, :],
                    in1=xdl[:, m, 3, :], op=OP.add)
            nc.sync.dma_start(
                xnew[:].rearrange("(m p) f -> p m f", m=MH, p=NP), xn[:])

    nc.finalize()
    if split:
        _split_multi_waits(nc)
    return nc


_CACHE = {}


def _get_program(widths):
    key = (widths["ACH"], widths["BCH"], widths["SW"], widths["FW"])
    if key not in _CACHE:
        _CACHE[key] = _build(widths)
    return _CACHE[key]


def kernel(**inputs) -> np.ndarray:
    from concourse.bass_utils import run_bass_kernel_spmd

    in_maps, widths, x = _prep(inputs)
    nc = _get_program(widths)
    res = run_bass_kernel_spmd(nc, in_maps, core_ids=list(range(C)))
    out = np.empty((1, T * N, F), np.float32)
    out[0, : (T - 1) * N] = x[N:]
    for c in range(C):
        out[0, (T - 1) * N + c * DSL:(T - 1) * N + (c + 1) * DSL] = \
            res.results[c]["xnew"]
    return out



# revision 4
# speedup vs baseline: 1.6942x; 1.6942x over previous
"""Trainium2 Bass kernel for nn_Net_12266426597866 (GNN message passing).

Strategy (8 NeuronCores, SPMD):
  - Dense normalized adjacency, column(dst)-sharded: each core builds its
    2000x250 slice of A_w^T (summed edge weights) ON PE by accumulating
    one-hot outer products (one-hots built by iota-compare on DVE, bf16),
    which also handles parallel-edge accumulation for free. An unweighted
    COUNT matrix C is built from the same one-hots. deg partials come
    straight from the src one-hots (AllGather overlaps the builds); the two
    ChebConvs collapse algebraically to two 9-column matmuls against the
    A_w slice.
  - Temporal convs on PE with block-Toeplitz packed weights.
  - Sparse-softmax attention: since h_st is scalar per node,
    prods[e,h] = kappa_h a_s a_d + s2_h a_s + s3_h a_d + s4_h. The global
    softmax denominator is computed WITHOUT per-edge gathers via
        Z_h = e^{s4} sum_v kappa^v/v! * F_v^T C G_v,   v = 0..2,
    (2nd-order Taylor of exp(kappa a_s a_d), |kappa a_s a_d| ~ 1e-3)
    with F_v(a) = a^v e^{s2 a} / G_v(a) = a^v e^{s3 a} as 24-column
    matmuls against the count matrix C.
  - Only the 2000 "prefix" edges (the dense div_op columns) need real
    per-edge a values: each core gathers its 250-edge slice (+ its own
    250-node a slice) with one small 768-index dma_gather; values ride the
    final AllGather together with the Z partials.
  - div_op matmul collapses to a 0/1 dedup'd adjacency B (min-clamped PE
    one-hot accumulation) times the 2000x8 matrix ea*diff*A[:2000].
  - Output: rows 0..5999 are exact copies of the input; device computes
    the 2000 x_new rows (250 per core).
"""

import os
import sys

sys.path.insert(0, "/opt/trn_rl_repo")

import numpy as np
import ml_dtypes

import concourse.bass as bass
import concourse.bacc as bacc
import concourse.mybir as mybir
import concourse.tile as tile
from concourse.masks import make_identity

F32 = mybir.dt.float32
BF16 = mybir.dt.bfloat16
I32 = mybir.dt.int32
I16 = mybir.dt.int16
AX = mybir.AxisListType
OP = mybir.AluOpType
ACT = mybir.ActivationFunctionType

# problem sizes
N, E, T, F = 2000, 32000, 4, 2
H, DK = 8, 16
C = 8                      # cores
NP, KT = 125, 16           # node tiling: n = p*KT + k  (p partition, k tile)
DSL = N // C               # 250 nodes (d-slice) per core
MH = 2                     # m-halves of d-slice (125 each)
CP = 256                   # padded column width of AW^T / B^T / C^T slices
EC = E // C                # own edge shard per core
NIDX = 768                 # dma_gather: prefix 125*2*2 + own-a 125*2 slots
ZW = 8 + 512               # z_in row: [Z partials | 512 gathered slots]
NV = 3                     # Taylor orders for the Z bilinear


def _ceil(a, b):
    return -(-a // b)


class _Packer:
    def __init__(self, dtype):
        self.cols = {}
        self.w = 0
        self.dtype = dtype

    def add(self, name, ncols):
        self.cols[name] = (self.w, self.w + ncols)
        self.w += ncols

    def alloc(self, rows=128, pad_to=8):
        w = _ceil(self.w, pad_to) * pad_to
        return np.zeros((rows, w), self.dtype), w


def _prep(inputs):
    """Host-side shard/table construction (index manipulation only)."""
    x = np.asarray(inputs["x_list"], np.float32)[0]          # (8000, 2)
    ei = np.asarray(inputs["edge_index"]).astype(np.int64)
    src, dst = ei[0], ei[1]
    ew = np.asarray(inputs["edge_attr"], np.float32)

    # ---- per-core grouped edge lists for the PE one-hot builds
    ach = bch = 1
    for c in range(C):
        lo = c * DSL
        sel = np.where((dst >= lo) & (dst < lo + DSL))[0]
        cnt = np.bincount(src[sel] % KT, minlength=KT)
        ach = max(ach, _ceil(int(cnt.max()), 128))
        selb = np.where((src >= lo) & (src < lo + DSL))[0]
        cntb = np.bincount(dst[selb] % KT, minlength=KT)
        bch = max(bch, _ceil(int(cntb.max()), 128))
    ACH, BCH = ach, bch
    AWW, BWW = KT * ACH, KT * BCH

    pb = _Packer(ml_dtypes.bfloat16)
    pb.add("awsp", AWW); pb.add("awdl", AWW); pb.add("awew", AWW)
    pb.add("bdp", BWW); pb.add("bsl", BWW)

    pf = _Packer(np.float32)
    pf.add("ea", KT); pf.add("xkt", KT * T * F); pf.add("xdl", MH * T * F)

    sp_ = _Packer(np.float32)
    for nm, w in [("w0", 2), ("w1", 2), ("w02T", 1), ("w12T", 1), ("b1", 1),
                  ("w1w", 1), ("w2wT", 2), ("w1b", 1), ("b2", 1), ("w2b", 2),
                  ("tc4b", 1), ("qka", 4), ("qkb", 4), ("hm", 8),
                  ("tc1wB", 64), ("tc2wB", 48), ("tc3wB", 32), ("tc4wB", 1),
                  ("tc1b", 1), ("tc2b", 1), ("tc3b", 1), ("msel", 2)]:
        sp_.add(nm, w)

    sm, SW = sp_.alloc()
    cs = sp_.cols

    def put(name, rows, arr):
        c0, c1 = cs[name]
        sm[:rows, c0:c1] = np.asarray(arr, np.float32).reshape(rows, c1 - c0)

    put("w0", 64, inputs["conv1_w0"])
    put("w1", 64, inputs["conv1_w1"])
    put("w02T", 64, np.asarray(inputs["conv2_w0"], np.float32).T)
    put("w12T", 64, np.asarray(inputs["conv2_w1"], np.float32).T)
    put("b1", 64, inputs["conv1_b"])
    put("w1w", 64, inputs["wout1_w"])
    put("w2wT", 64, np.asarray(inputs["wout2_w"], np.float32).T)
    put("w1b", 64, inputs["wout1_b"])
    put("b2", 1, inputs["conv2_b"])
    put("w2b", 1, np.asarray(inputs["wout2_b"], np.float32).reshape(1, 2))
    put("tc4b", 1, inputs["tc4_b"])
    qw = np.asarray(inputs["q_w"], np.float32)[:, 0]
    qb = np.asarray(inputs["q_b"], np.float32)
    kw = np.asarray(inputs["k_w"], np.float32)[:, 0]
    kb = np.asarray(inputs["k_b"], np.float32)
    put("qka", 128, np.stack([qw, qw, qb, qb], 1))
    put("qkb", 128, np.stack([kw, kb, kw, kb], 1))
    hm = (np.arange(128)[:, None] // DK == np.arange(H)[None, :]).astype(np.float32)
    put("hm", 128, hm)
    tc1 = np.asarray(inputs["tc1_w"], np.float32)
    tc2 = np.asarray(inputs["tc2_w"], np.float32)
    tc3 = np.asarray(inputs["tc3_w"], np.float32)
    tc4 = np.asarray(inputs["tc4_w"], np.float32)
    w1B = np.zeros((4, 64), np.float32)
    for t in range(4):
        for kk in range(3):
            r = t + kk - 1
            if 0 <= r < 4:
                w1B[r, t * 16:(t + 1) * 16] = tc1[:, 0, 0, kk]
    w2B = np.zeros((64, 48), np.float32)
    for t2 in range(3):
        for kk in range(2):
            w2B[(t2 + kk) * 16:(t2 + kk + 1) * 16, t2 * 16:(t2 + 1) * 16] = \
                tc2[:, :, 0, kk].T
    w3B = np.zeros((48, 32), np.float32)
    for t3 in range(2):
        for kk in range(2):
            w3B[(t3 + kk) * 16:(t3 + kk + 1) * 16, t3 * 16:(t3 + 1) * 16] = \
                tc3[:, :, 0, kk].T
    w4B = np.zeros((32, 1), np.float32)
    for kk in range(2):
        w4B[kk * 16:(kk + 1) * 16, 0] = tc4[0, :, 0, kk]
    put("tc1wB", 4, w1B); put("tc2wB", 64, w2B)
    put("tc3wB", 48, w3B); put("tc4wB", 32, w4B)
    put("tc1b", 64, np.tile(np.asarray(inputs["tc1_b"], np.float32), T))
    put("tc2b", 48, np.tile(np.asarray(inputs["tc2_b"], np.float32), 3))
    put("tc3b", 32, np.tile(np.asarray(inputs["tc3_b"], np.float32), 2))
    msel = np.zeros((16, 2), np.float32)
    msel[np.arange(16), np.arange(16) // C] = 1.0
    put("msel", 16, msel)

    pidx = np.arange(NP)
    jj = (pidx[:, None] * KT + np.arange(KT)[None, :])       # j or n = p*16+k

    in_maps = []
    for c in range(C):
        fb, FWB = pb.alloc(rows=128)
        ff, FWF = pf.alloc(rows=128)
        bc, fc = pb.cols, pf.cols

        def bput(name, arr2d):
            c0, c1 = bc[name]
            fb[: arr2d.shape[0], c0:c1] = arr2d.astype(ml_dtypes.bfloat16)

        def fput(name, arr2d):
            c0, c1 = fc[name]
            ff[: arr2d.shape[0], c0:c1] = arr2d

        lo = c * DSL
        sel = np.where((dst >= lo) & (dst < lo + DSL))[0]
        sp = np.full((KT, ACH * 128), -1.0, np.float32)
        dl = np.full((KT, ACH * 128), -1.0, np.float32)
        wv = np.zeros((KT, ACH * 128), np.float32)
        for k in range(KT):
            e = sel[src[sel] % KT == k]
            sp[k, :len(e)] = src[e] // KT
            dl[k, :len(e)] = dst[e] - lo
            wv[k, :len(e)] = ew[e]
        # layout [128 rows, (k, ch) cols]: row = edge-in-chunk
        bput("awsp", sp.reshape(KT * ACH, 128).T)
        bput("awdl", dl.reshape(KT * ACH, 128).T)
        bput("awew", wv.reshape(KT * ACH, 128).T)
        selb = np.where((src >= lo) & (src < lo + DSL))[0]
        dp = np.full((KT, BCH * 128), -1.0, np.float32)
        sl = np.full((KT, BCH * 128), -1.0, np.float32)
        for k in range(KT):
            e = selb[dst[selb] % KT == k]
            dp[k, :len(e)] = dst[e] // KT
            sl[k, :len(e)] = src[e] - lo
        bput("bdp", dp.reshape(KT * BCH, 128).T)
        bput("bsl", sl.reshape(KT * BCH, 128).T)

        fput("ea", ew[jj])
        xk = np.zeros((NP, KT, T, F), np.float32)
        for t in range(T):
            xk[:, :, t, :] = x[t * N + jj]
        fput("xkt", xk.reshape(NP, KT * T * F))
        xd = np.zeros((NP, MH, T, F), np.float32)
        for m in range(MH):
            for t in range(T):
                xd[:, m, t, :] = x[t * N + c * DSL + m * NP + pidx]
        fput("xdl", xd.reshape(NP, MH * T * F))

        # ---- int32 tables: deg d-local run-gather offsets (row q = (m, r))
        it = np.zeros((16, 8), np.int32)
        for q in range(16):
            m, r = q // C, q % C
            it[q, 0] = r * N + c * DSL + m * NP
        # ---- dma_gather index list: slot i -> out[i%128, i//128, 0]
        #   i = v*256 + h*128 + p : a[src/dst of prefix edge j=p*16+(2c+h)]
        #   i = 512 + m*128 + p   : a[own d-slice node c*DSL+m*125+p]
        gl = np.zeros(NIDX, np.int64)
        for v in range(2):
            ends = src if v == 0 else dst
            for h in range(2):
                j = pidx * KT + (2 * c + h)
                gl[v * 256 + h * 128: v * 256 + h * 128 + NP] = ends[j]
        for m in range(MH):
            gl[512 + m * 128: 512 + m * 128 + NP] = c * DSL + m * NP + pidx
        g16t = np.zeros((16, NIDX // 16), np.int16)
        ii = np.arange(NIDX)
        g16t[ii % 16, ii // 16] = gl.astype(np.int16)
        g16t = np.tile(g16t, (8, 1))
        in_maps.append({"smalls": sm, "ftb": fb, "ftf": ff, "itabs": it,
                        "g16": g16t})

    widths = dict(ACH=ACH, BCH=BCH, SW=SW, FWB=FWB, FWF=FWF,
                  bcols=dict(pb.cols), fcols=dict(pf.cols),
                  scols=dict(sp_.cols))
    return in_maps, widths, x


def _split_multi_waits(nc):
    """Walrus codegen in this container accepts only one inline sync wait per
    instruction; hoist extras into standalone EventSemaphore waits."""
    for func in nc.m.functions:
        for bb in func.blocks:
            out = []
            for inst in bb.instructions:
                si = inst.sync_info
                waits = list(si.on_wait) if (si is not None and si.on_wait) else []
                if len(waits) > 1:
                    for w in waits[:-1]:
                        out.append(mybir.InstEventSemaphore(
                            name=nc.get_next_instruction_name(),
                            engine=inst.engine, ins=[], outs=[],
                            sync_info=mybir.SyncInfo(on_wait=[w], on_update=[])))
                    inst.sync_info = mybir.SyncInfo(on_wait=[waits[-1]],
                                                    on_update=list(si.on_update))
                out.append(inst)
            bb.instructions = out


def _build(w, split=True):
    """Construct the SPMD Bass program (identical across cores)."""
    nc = bacc.Bacc(None, num_devices=C)
    bc, fc, sc = w["bcols"], w["fcols"], w["scols"]
    ACH, BCH = w["ACH"], w["BCH"]

    smalls = nc.declare_dram_parameter("smalls", [128, w["SW"]], F32, isOutput=False)
    ftb = nc.declare_dram_parameter("ftb", [128, w["FWB"]], BF16, isOutput=False)
    ftf = nc.declare_dram_parameter("ftf", [128, w["FWF"]], F32, isOutput=False)
    itabs = nc.declare_dram_parameter("itabs", [16, 8], I32, isOutput=False)
    g16 = nc.declare_dram_parameter("g16", [128, NIDX // 16], I16, isOutput=False)
    xnew = nc.declare_dram_parameter("xnew", [DSL, F], F32, isOutput=True)

    deg_in = nc.dram_tensor("deg_in", [NP, KT], F32)
    deg_out = nc.dram_tensor("deg_out", [C, N], F32, addr_space="Shared")
    t1_in = nc.dram_tensor("t1_in", [DSL, 8], F32)
    t1_out = nc.dram_tensor("t1_out", [N, 8], F32, addr_space="Shared")
    a_in = nc.dram_tensor("a_in", [1, DSL], F32)
    a_out = nc.dram_tensor("a_out", [1, N], F32, addr_space="Shared")
    z_in = nc.dram_tensor("z_in", [1, ZW], F32)
    z_out = nc.dram_tensor("z_out", [C, ZW], F32, addr_space="Shared")
    arep = nc.dram_tensor("arep", [N, 64], F32)
    RG = [list(range(C))]

    IOA = bass.IndirectOffsetOnAxis

    with tile.TileContext(nc) as tc:
        with (
            tc.tile_pool(name="sb", bufs=1) as sb,
            tc.tile_pool(name="psb", bufs=2, space="PSUM") as psb,
            tc.tile_pool(name="ps9", bufs=2, space="PSUM") as ps9,
            tc.tile_pool(name="psg", bufs=2, space="PSUM") as psg,
            tc.tile_pool(name="pst", bufs=2, space="PSUM") as pst,
        ):
            def bsl_(name):
                c0, c1 = bc[name]
                return fb_sb[:, c0:c1]

            def fsl(name):
                c0, c1 = fc[name]
                return ff_sb[:, c0:c1]

            def ssl(name, rows=64):
                c0, c1 = sc[name]
                return sm_sb[:rows, c0:c1]

            # ---------- stage inputs
            sm_sb = sb.tile([128, w["SW"]], F32, name="sm")
            fb_sb = sb.tile([128, w["FWB"]], BF16, name="fb")
            ff_sb = sb.tile([128, w["FWF"]], F32, name="ff")
            it_sb = sb.tile([16, 8], I32, name="it")
            g16_sb = sb.tile([128, NIDX // 16], I16, name="g16s")
            nc.sync.dma_start(sm_sb[:], smalls[:])
            nc.sync.dma_start(fb_sb[:], ftb[:])
            nc.sync.dma_start(ff_sb[:], ftf[:])
            nc.sync.dma_start(it_sb[:], itabs[:])
            nc.sync.dma_start(g16_sb[:], g16[:])
            id_sb = sb.tile([128, 128], F32, name="idm")
            make_identity(nc, id_sb[:])
            ones = sb.tile([128, 1], F32, name="ones")
            nc.gpsimd.memset(ones[:], 1.0)
            onesb = sb.tile([128, 1], BF16, name="onesb")
            nc.gpsimd.memset(onesb[:], 1.0)
            ones_mat = sb.tile([128, 128], F32, name="onesm")
            nc.gpsimd.memset(ones_mat[:], 1.0)
            iot = sb.tile([128, CP], BF16, name="iot")
            nc.gpsimd.iota(iot[:], pattern=[[1, CP]], channel_multiplier=0,
                           allow_small_or_imprecise_dtypes=True)

            # ---------- attention coefficients [kappa|s2|s3|s4] per head
            qp = sb.tile([128, 4], F32, name="qp")
            nc.vector.tensor_tensor(out=qp[:], in0=ssl("qka", 128),
                                    in1=ssl("qkb", 128), op=OP.mult)
            qh = sb.tile([128, 4, H], F32, name="qh")
            nc.vector.tensor_tensor(
                out=qh[:], in0=qp[:].unsqueeze(2).broadcast_to([128, 4, H]),
                in1=ssl("hm", 128).unsqueeze(1).broadcast_to([128, 4, H]),
                op=OP.mult)
            cfp = pst.tile([128, 4 * H], F32, tag="t")
            nc.tensor.matmul(out=cfp[:], lhsT=ones_mat[:], rhs=qh[:],
                             start=True, stop=True)
            cb = sb.tile([128, 4 * H], F32, name="cb")
            nc.scalar.mul(cb[:], cfp[:], 0.25)            # /sqrt(DK)=4

            # ---------- combined cheb weight row:
            # [M0_0, M0_1, M1_0, M1_1, M2_0, M2_1, alpha, beta]
            w02b2 = ssl("w02T").to_broadcast([64, 2])
            w12b2 = ssl("w12T").to_broadcast([64, 2])
            p8 = sb.tile([65, 8], F32, name="p8")
            nc.gpsimd.memset(p8[:], 0.0)
            scr2 = sb.tile([64, 2], F32, name="scr2")
            nc.vector.tensor_tensor(out=p8[0:64, 0:2], in0=ssl("w0"), in1=w02b2,
                                    op=OP.mult)
            nc.vector.tensor_tensor(out=p8[0:64, 2:4], in0=ssl("w1"), in1=w02b2,
                                    op=OP.mult)
            nc.vector.tensor_tensor(out=scr2[:], in0=ssl("w0"), in1=w12b2,
                                    op=OP.mult)
            nc.vector.tensor_tensor(out=p8[0:64, 2:4], in0=p8[0:64, 2:4],
                                    in1=scr2[:], op=OP.add)
            nc.vector.tensor_tensor(out=p8[0:64, 4:6], in0=ssl("w1"), in1=w12b2,
                                    op=OP.mult)
            nc.vector.tensor_tensor(out=p8[0:64, 6:7], in0=ssl("b1"),
                                    in1=ssl("w02T"), op=OP.mult)
            nc.vector.tensor_tensor(out=p8[0:64, 7:8], in0=ssl("b1"),
                                    in1=ssl("w12T"), op=OP.mult)
            nc.vector.tensor_copy(p8[64:65, 6:7], ssl("b2", 1))
            mrp = pst.tile([128, 8], F32, tag="t")
            nc.tensor.matmul(out=mrp[:], lhsT=ones_mat[0:65, :], rhs=p8[:],
                             start=True, stop=True)
            mb = sb.tile([128, 8], F32, name="mb")
            nc.vector.tensor_copy(mb[:], mrp[:])

            # ---------- output weight consts
            vp = pst.tile([1, 2], F32, tag="t")
            nc.tensor.matmul(out=vp[:], lhsT=ssl("w1w"), rhs=ssl("w2wT"),
                             start=True, stop=True)
            cstp = pst.tile([1, 2], F32, tag="t")
            nc.tensor.matmul(out=cstp[:], lhsT=ssl("w1b"), rhs=ssl("w2wT"),
                             start=True, stop=True)
            vc = sb.tile([1, 4], F32, name="vc")
            nc.scalar.mul(vc[0:1, 0:2], vp[:], 1.0 / H)
            nc.vector.tensor_tensor(out=vc[0:1, 2:4], in0=cstp[:],
                                    in1=ssl("w2b", 1), op=OP.add)
            vcp = pst.tile([128, 4], F32, tag="t")
            nc.tensor.matmul(out=vcp[:], lhsT=ones_mat[0:1, :], rhs=vc[:],
                             start=True, stop=True)
            vcb = sb.tile([128, 4], F32, name="vcb")
            nc.vector.tensor_copy(vcb[:], vcp[:])

            # ---------- src-side one-hots (raw + ew-scaled) for AW / C / deg
            nch_a = KT * ACH
            os_r = sb.tile([128, nch_a, NP], BF16, name="osr")
            nc.vector.tensor_tensor(
                out=os_r[:],
                in0=iot[:, 0:NP].unsqueeze(1).to_broadcast([128, nch_a, NP]),
                in1=bsl_("awsp").unsqueeze(2).to_broadcast([128, nch_a, NP]),
                op=OP.is_equal)
            os_s = sb.tile([128, nch_a, NP], BF16, name="oss")
            nc.vector.tensor_tensor(
                out=os_s[:], in0=os_r[:],
                in1=bsl_("awew").unsqueeze(2).to_broadcast([128, nch_a, NP]),
                op=OP.mult)

            # ---------- early deg partials: deg[p,k] = sum_slots os_s
            dps = pst.tile([NP, KT], F32, tag="t")
            for k in range(KT):
                for ch in range(ACH):
                    nc.tensor.matmul(out=dps[:, k:k + 1],
                                     lhsT=os_s[:, k * ACH + ch, :],
                                     rhs=onesb[:],
                                     start=(ch == 0), stop=(ch == ACH - 1))
            degp = sb.tile([NP, KT], F32, name="degp")
            nc.vector.tensor_copy(degp[:], dps[:])
            nc.sync.dma_start(deg_in[:], degp[:])
            nc.gpsimd.collective_compute(
                "AllGather", OP.bypass, replica_groups=RG,
                ins=[deg_in[:]], outs=[deg_out[:]])

            # ---------- dst one-hots; build C then AW by accumulation
            od_a = sb.tile([128, nch_a, CP], BF16, name="oda")
            nc.vector.tensor_tensor(
                out=od_a[:],
                in0=iot[:].unsqueeze(1).to_broadcast([128, nch_a, CP]),
                in1=bsl_("awdl").unsqueeze(2).to_broadcast([128, nch_a, CP]),
                op=OP.is_equal)

            c_sb = sb.tile([NP, KT, CP], BF16, name="cmat")
            for k in range(KT):
                ps = psb.tile([NP, CP], F32, tag="bld")
                for ch in range(ACH):
                    nc.tensor.matmul(out=ps[:], lhsT=os_r[:, k * ACH + ch, :],
                                     rhs=od_a[:, k * ACH + ch, :],
                                     start=(ch == 0), stop=(ch == ACH - 1))
                nc.vector.tensor_copy(c_sb[:, k, :], ps[:])

            aw_sb = sb.tile([NP, KT, CP], BF16, name="aw")
            for k in range(KT):
                ps = psb.tile([NP, CP], F32, tag="bld")
                for ch in range(ACH):
                    nc.tensor.matmul(out=ps[:], lhsT=os_s[:, k * ACH + ch, :],
                                     rhs=od_a[:, k * ACH + ch, :],
                                     start=(ch == 0), stop=(ch == ACH - 1))
                nc.vector.tensor_copy(aw_sb[:, k, :], ps[:])

            # ---------- build B^T (bf16, min-clamped for dedup)
            nch_b = KT * BCH
            os_b = sb.tile([128, nch_b, NP], BF16, name="osb")
            nc.vector.tensor_tensor(
                out=os_b[:],
                in0=iot[:, 0:NP].unsqueeze(1).to_broadcast([128, nch_b, NP]),
                in1=bsl_("bdp").unsqueeze(2).to_broadcast([128, nch_b, NP]),
                op=OP.is_equal)
            od_b = sb.tile([128, nch_b, CP], BF16, name="odb")
            nc.vector.tensor_tensor(
                out=od_b[:],
                in0=iot[:].unsqueeze(1).to_broadcast([128, nch_b, CP]),
                in1=bsl_("bsl").unsqueeze(2).to_broadcast([128, nch_b, CP]),
                op=OP.is_equal)
            b_sb = sb.tile([NP, KT, CP], BF16, name="bsb")
            for k in range(KT):
                ps = psb.tile([NP, CP], F32, tag="bld")
                for ch in range(BCH):
                    nc.tensor.matmul(out=ps[:], lhsT=os_b[:, k * BCH + ch, :],
                                     rhs=od_b[:, k * BCH + ch, :],
                                     start=(ch == 0), stop=(ch == BCH - 1))
                nc.vector.tensor_scalar(out=b_sb[:, k, :], in0=ps[:],
                                        scalar1=1.0, scalar2=None, op0=OP.min)

            # ---------- deg post-AG: full-node + d-local inv-sqrt scalings
            degs = sb.tile([NP, C, KT], F32, name="degs")
            nc.sync.dma_start(
                degs[:], deg_out[:].rearrange("r (p k) -> p r k", p=NP))
            degkt = sb.tile([NP, KT], F32, name="degkt")
            nc.vector.reduce_sum(
                out=degkt[:], in_=degs[:].rearrange("p r k -> p k r"), axis=AX.X)

            sq = sb.tile([NP, KT], F32, name="sq")
            nc.scalar.sqrt(sq[:], degkt[:])
            rc = sb.tile([NP, KT], F32, name="rc")
            nc.vector.reciprocal(rc[:], sq[:])
            msk = sb.tile([NP, KT], F32, name="msk")
            nc.vector.tensor_scalar(out=msk[:], in0=degkt[:], scalar1=0.0,
                                    scalar2=None, op0=OP.is_gt)
            is_kt = sb.tile([NP, KT], F32, name="iskt")
            nc.vector.tensor_tensor(out=is_kt[:], in0=rc[:], in1=msk[:], op=OP.mult)
            is2 = sb.tile([NP, KT], F32, name="is2")
            nc.vector.scalar_tensor_tensor(out=is2[:], in0=is_kt[:], scalar=-1.0,
                                           in1=is_kt[:], op0=OP.mult, op1=OP.mult)

            # ---------- d-local -inv-sqrt via run-gather of deg_out partials
            dd8 = sb.tile([16, NP], F32, name="dd8")
            nc.gpsimd.indirect_dma_start(
                out=dd8[:], out_offset=None,
                in_=deg_out[:], in_offset=IOA(ap=it_sb[:, 0:1], axis=1))
            ddp = pst.tile([2, NP], F32, tag="t")
            nc.tensor.matmul(out=ddp[:], lhsT=ssl("msel", 16), rhs=dd8[:],
                             start=True, stop=True)
            ddsb = sb.tile([2, NP], F32, name="ddsb")
            nc.vector.tensor_copy(ddsb[:], ddp[:])
            ddtp = pst.tile([NP, 2], F32, tag="t")
            nc.tensor.transpose(out=ddtp[:], in_=ddsb[:],
                                identity=id_sb[:2, :2])
            ddl = sb.tile([NP, MH], F32, name="ddl")
            nc.vector.tensor_copy(ddl[:], ddtp[:])
            sqd = sb.tile([NP, MH], F32, name="sqd")
            nc.scalar.sqrt(sqd[:], ddl[:])
            rcd = sb.tile([NP, MH], F32, name="rcd")
            nc.vector.reciprocal(rcd[:], sqd[:])
            mskd = sb.tile([NP, MH], F32, name="mskd")
            nc.vector.tensor_scalar(out=mskd[:], in0=ddl[:], scalar1=0.0,
                                    scalar2=None, op0=OP.is_gt)
            nisdl = sb.tile([NP, MH], F32, name="nisdl")
            nc.vector.scalar_tensor_tensor(out=nisdl[:], in0=rcd[:], scalar=-1.0,
                                           in1=mskd[:], op0=OP.mult, op1=OP.mult)

            # ---------- t1 = A_w @ (S x)   (9th col: S*ones for row sums)
            xkt = fsl("xkt")[:NP, :].rearrange("p (k c) -> p k c", k=KT)
            rhs1 = sb.tile([NP, KT, 9], BF16, name="rhs1")
            nc.vector.tensor_tensor(
                out=rhs1[:, :, 0:8], in0=xkt,
                in1=is_kt[:].unsqueeze(2).to_broadcast([NP, KT, 8]), op=OP.mult)
            nc.vector.tensor_copy(rhs1[:, :, 8], is_kt[:])

            ta_sb = sb.tile([NP, MH, 9], F32, name="ta")
            for m in range(MH):
                tp = ps9.tile([NP, 9], F32, tag="mm9")
                for k in range(KT):
                    nc.tensor.matmul(
                        out=tp[:], lhsT=aw_sb[:, k, m * NP:(m + 1) * NP],
                        rhs=rhs1[:, k, :], start=(k == 0), stop=(k == KT - 1))
                nc.vector.tensor_copy(ta_sb[:, m, :], tp[:])

            nc.sync.dma_start(
                t1_in[:].rearrange("(m p) c -> p m c", m=MH),
                ta_sb[:, :, 0:8])
            nc.gpsimd.collective_compute(
                "AllGather", OP.bypass, replica_groups=RG,
                ins=[t1_in[:]], outs=[t1_out[:]])
            t1f = sb.tile([NP, KT, 8], F32, name="t1f")
            nc.sync.dma_start(
                t1f[:], t1_out[:].rearrange("(p k) c -> p k c", p=NP))

            # ---------- u = A_w @ ((-S^2) ta_full)
            rhsu = sb.tile([NP, KT, 8], BF16, name="rhsu")
            nc.vector.tensor_tensor(
                out=rhsu[:], in0=t1f[:],
                in1=is2[:].unsqueeze(2).to_broadcast([NP, KT, 8]), op=OP.mult)
            ua_sb = sb.tile([NP, MH, 8], F32, name="ua")
            for m in range(MH):
                up = ps9.tile([NP, 8], F32, tag="mm9")
                for k in range(KT):
                    nc.tensor.matmul(
                        out=up[:], lhsT=aw_sb[:, k, m * NP:(m + 1) * NP],
                        rhs=rhsu[:, k, :], start=(k == 0), stop=(k == KT - 1))
                nc.vector.tensor_copy(ua_sb[:, m, :], up[:])

            # ---------- h2 = Xterm + (-is_d) * Q
            xdl = fsl("xdl")[:NP, :].rearrange("p (m t f) -> p m t f", m=MH, t=T)
            xterm = sb.tile([NP, MH, T], F32, name="xterm")
            nc.vector.tensor_scalar(
                out=xterm[:], in0=xdl[:, :, :, 0],
                scalar1=mb[:NP, 0:1], scalar2=None, op0=OP.mult)
            nc.vector.scalar_tensor_tensor(
                out=xterm[:], in0=xdl[:, :, :, 1],
                scalar=mb[:NP, 1:2],
                in1=xterm[:], op0=OP.mult, op1=OP.add)
            nc.vector.tensor_scalar(
                out=xterm[:], in0=xterm[:],
                scalar1=mb[:NP, 6:7], scalar2=None,
                op0=OP.add)

            qq = sb.tile([NP, MH, T], F32, name="qq")
            nc.vector.tensor_scalar(
                out=qq[:], in0=ta_sb[:, :, 0:8:2],
                scalar1=mb[:NP, 2:3], scalar2=None,
                op0=OP.mult)
            nc.vector.scalar_tensor_tensor(
                out=qq[:], in0=ta_sb[:, :, 1:9:2],
                scalar=mb[:NP, 3:4],
                in1=qq[:], op0=OP.mult, op1=OP.add)
            nc.vector.scalar_tensor_tensor(
                out=qq[:], in0=ua_sb[:, :, 0:8:2],
                scalar=mb[:NP, 4:5],
                in1=qq[:], op0=OP.mult, op1=OP.add)
            nc.vector.scalar_tensor_tensor(
                out=qq[:], in0=ua_sb[:, :, 1:8:2],
                scalar=mb[:NP, 5:6],
                in1=qq[:], op0=OP.mult, op1=OP.add)
            nc.vector.scalar_tensor_tensor(
                out=qq[:], in0=ta_sb[:, :, 8:9].to_broadcast([NP, MH, T]),
                scalar=mb[:NP, 7:8],
                in1=qq[:], op0=OP.mult, op1=OP.add)

            h2 = sb.tile([NP, MH, T], F32, name="h2")
            for m in range(MH):
                nc.vector.scalar_tensor_tensor(
                    out=h2[:, m, :], in0=qq[:, m, :],
                    scalar=nisdl[:, m:m + 1],
                    in1=xterm[:, m, :], op0=OP.mult, op1=OP.add)

            # ---------- temporal convs ((tau, ch) on partitions, nodes free)
            hc4 = sb.tile([T, MH * NP], F32, name="hc4")
            for m in range(MH):
                tps = pst.tile([T, NP], F32, tag="t")
                nc.tensor.transpose(out=tps[:], in_=h2[:, m, :],
                                    identity=id_sb[:NP, :NP])
                nc.vector.tensor_copy(hc4[:, m * NP:(m + 1) * NP], tps[:])

            s1 = sb.tile([64, MH * NP], F32, name="s1t")
            cp1 = psb.tile([64, MH * NP], F32, tag="bld")
            nc.tensor.matmul(out=cp1[:], lhsT=ssl("tc1wB", 4),
                             rhs=hc4[:], start=True, stop=True)
            nc.vector.tensor_scalar(out=s1[:], in0=cp1[:],
                                    scalar1=ssl("tc1b"), scalar2=None,
                                    op0=OP.add)
            s2t = sb.tile([48, MH * NP], F32, name="s2t")
            cp2 = psb.tile([48, MH * NP], F32, tag="bld")
            nc.tensor.matmul(out=cp2[:], lhsT=ssl("tc2wB"),
                             rhs=s1[:], start=True, stop=True)
            nc.vector.tensor_scalar(out=s2t[:], in0=cp2[:],
                                    scalar1=ssl("tc2b", 48), scalar2=None,
                                    op0=OP.add)
            s3t = sb.tile([32, MH * NP], F32, name="s3t")
            cp3 = psb.tile([32, MH * NP], F32, tag="bld")
            nc.tensor.matmul(out=cp3[:], lhsT=ssl("tc3wB", 48),
                             rhs=s2t[:], start=True, stop=True)
            nc.vector.tensor_scalar(out=s3t[:], in0=cp3[:],
                                    scalar1=ssl("tc3b", 32), scalar2=None,
                                    op0=OP.add)
            ap4 = psb.tile([1, MH * NP], F32, tag="bld")
            nc.tensor.matmul(out=ap4[:], lhsT=ssl("tc4wB", 32),
                             rhs=s3t[:], start=True, stop=True)
            a_sb = sb.tile([1, MH * NP], F32, name="asb")
            nc.vector.tensor_scalar(out=a_sb[:], in0=ap4[:],
                                    scalar1=ssl("tc4b", 1), scalar2=None,
                                    op0=OP.add)
            nc.sync.dma_start(a_in[:], a_sb[:])
            nc.gpsimd.collective_compute(
                "AllGather", OP.bypass, replica_groups=RG,
                ins=[a_in[:]], outs=[a_out[:]])

            # ---------- replicated a table (2000, 64) + one small dma_gather
            akt = sb.tile([NP, KT], F32, name="akt")
            nc.sync.dma_start(akt[:],
                              a_out[:].rearrange("o (p k) -> p (o k)", p=NP))
            arr = sb.tile([NP, KT, 64], F32, name="arr")
            nc.vector.tensor_copy(
                arr[:], akt[:].unsqueeze(2).to_broadcast([NP, KT, 64]))
            nc.sync.dma_start(
                arep[:].rearrange("(p k) c -> p k c", p=NP), arr[:])
            gat = sb.tile([128, NIDX // 128, 64], F32, name="gat")
            nc.gpsimd.dma_gather(
                out_ap=gat[:], in_ap=arep[:],
                idxs_ap=g16_sb[:],
                num_idxs=NIDX, num_idxs_reg=NIDX, elem_size=64)

            # ---------- Z via bilinear: phi columns (src side, (p,k) nodes)
            phs = sb.tile([NP, KT, H], F32, name="phs")
            nc.vector.tensor_tensor(
                out=phs[:], in0=akt[:].unsqueeze(2).to_broadcast([NP, KT, H]),
                in1=cb[:NP, H:2 * H].unsqueeze(1).to_broadcast([NP, KT, H]),
                op=OP.mult)
            phf = sb.tile([NP, KT, NV * H], F32, name="phf")
            nc.scalar.activation(phf[:, :, 0:H], phs[:], ACT.Exp)
            nc.vector.tensor_tensor(
                out=phf[:, :, H:2 * H], in0=phf[:, :, 0:H],
                in1=akt[:].unsqueeze(2).to_broadcast([NP, KT, H]), op=OP.mult)
            nc.vector.tensor_tensor(
                out=phf[:, :, 2 * H:3 * H], in0=phf[:, :, H:2 * H],
                in1=akt[:].unsqueeze(2).to_broadcast([NP, KT, H]), op=OP.mult)
            phi_b = sb.tile([NP, KT, NV * H], BF16, name="phib")
            nc.vector.tensor_copy(phi_b[:], phf[:])

            # ---------- psi columns (dst side, own d-slice from gather)
            adl = gat[:NP, 4:6, 0]                       # [125, 2] own a
            pss = sb.tile([NP, MH, H], F32, name="pss")
            nc.vector.tensor_tensor(
                out=pss[:], in0=adl.unsqueeze(2).to_broadcast([NP, MH, H]),
                in1=cb[:NP, 2 * H:3 * H].unsqueeze(1).to_broadcast([NP, MH, H]),
                op=OP.mult)
            psf = sb.tile([NP, MH, NV * H], F32, name="psf")
            nc.scalar.activation(psf[:, :, 0:H], pss[:], ACT.Exp)
            nc.vector.tensor_tensor(
                out=psf[:, :, H:2 * H], in0=psf[:, :, 0:H],
                in1=adl.unsqueeze(2).to_broadcast([NP, MH, H]), op=OP.mult)
            nc.vector.tensor_tensor(
                out=psf[:, :, 2 * H:3 * H], in0=psf[:, :, H:2 * H],
                in1=adl.unsqueeze(2).to_broadcast([NP, MH, H]), op=OP.mult)

            # ---------- Z partial = sum_dstLocal psi * (C^T phi)
            tzp = pst.tile([1, NV * H], F32, tag="t")
            for m in range(MH):
                cpz = ps9.tile([NP, NV * H], F32, tag="mm9")
                for k in range(KT):
                    nc.tensor.matmul(
                        out=cpz[:], lhsT=c_sb[:, k, m * NP:(m + 1) * NP],
                        rhs=phi_b[:, k, :], start=(k == 0), stop=(k == KT - 1))
                ctf = sb.tile([NP, NV * H], F32, name=f"ctf{m}")
                nc.vector.tensor_copy(ctf[:], cpz[:])
                prod = sb.tile([NP, NV * H], F32, name=f"prod{m}")
                nc.vector.tensor_tensor(out=prod[:], in0=ctf[:],
                                        in1=psf[:, m, :], op=OP.mult)
                nc.tensor.matmul(out=tzp[:], lhsT=ones[:NP, :], rhs=prod[:],
                                 start=(m == 0), stop=(m == MH - 1))
            tsb = sb.tile([1, NV * H], F32, name="tsb")
            nc.vector.tensor_copy(tsb[:], tzp[:])

            # Z = e^{s4} (T0 + kappa T1 + kappa^2/2 T2)
            e4 = sb.tile([1, H], F32, name="e4")
            nc.scalar.activation(e4[:], cb[0:1, 3 * H:4 * H], ACT.Exp)
            zc = sb.tile([1, H], F32, name="zc")
            nc.vector.scalar_tensor_tensor(
                out=zc[:], in0=cb[0:1, 0:H], scalar=0.5,
                in1=tsb[0:1, 2 * H:3 * H], op0=OP.mult, op1=OP.mult)
            nc.vector.tensor_tensor(out=zc[:], in0=zc[:],
                                    in1=tsb[0:1, H:2 * H], op=OP.add)
            nc.vector.tensor_tensor(out=zc[:], in0=zc[:],
                                    in1=cb[0:1, 0:H], op=OP.mult)
            nc.vector.tensor_tensor(out=zc[:], in0=zc[:],
                                    in1=tsb[0:1, 0:H], op=OP.add)
            zsb = sb.tile([1, H], F32, name="zsb")
            nc.vector.tensor_tensor(out=zsb[:], in0=zc[:], in1=e4[:], op=OP.mult)

            # ---------- AG4: [Z partial | gathered prefix a values]
            nc.sync.dma_start(z_in[0:1, 0:H], zsb[:])
            nc.sync.dma_start(
                z_in[0:1, H:ZW].rearrange("o (q p) -> p (o q)", q=4, p=128),
                gat[:, 0:4, 0])
            nc.gpsimd.collective_compute(
                "AllGather", OP.bypass, replica_groups=RG,
                ins=[z_in[:]], outs=[z_out[:]])

            z8 = sb.tile([C, H], F32, name="z8")
            nc.sync.dma_start(z8[:], z_out[:, 0:H])
            ztp = pst.tile([128, H], F32, tag="t")
            nc.tensor.matmul(out=ztp[:], lhsT=ones_mat[0:C, :], rhs=z8[:],
                             start=True, stop=True)
            zf = sb.tile([128, H], F32, name="zf")
            nc.vector.tensor_copy(zf[:], ztp[:])
            rzb = sb.tile([128, H], F32, name="rzb")
            nc.vector.reciprocal(rzb[:], zf[:])

            # ---------- prefix edges -> gxe rows (j = p*16+k, k = 2c+h)
            asp = sb.tile([128, KT], F32, name="asp")
            adp = sb.tile([128, KT], F32, name="adp")
            for hh in range(2):
                o = H + hh * 128
                nc.sync.dma_start(
                    asp[:].rearrange("p (c h) -> p c h", h=2)[:, :, hh],
                    z_out[:, o:o + 128].rearrange("c p -> p c"))
                nc.sync.dma_start(
                    adp[:].rearrange("p (c h) -> p c h", h=2)[:, :, hh],
                    z_out[:, o + 256:o + 384].rearrange("c p -> p c"))

            def heads_pre(row):
                return (cb[:NP, row * H:(row + 1) * H].unsqueeze(1)
                        .broadcast_to([NP, KT, H]))

            aspv = asp[:NP, :]
            adpv = adp[:NP, :]
            asadp = sb.tile([NP, KT], F32, name="asadp")
            nc.vector.tensor_tensor(out=asadp[:], in0=aspv, in1=adpv, op=OP.mult)
            ppre = sb.tile([NP, KT, H], F32, name="ppre")
            nc.vector.tensor_tensor(
                out=ppre[:], in0=asadp[:].unsqueeze(2).to_broadcast([NP, KT, H]),
                in1=heads_pre(0), op=OP.mult)
            tbp = sb.tile([NP, KT, H], F32, name="tbp")
            nc.vector.tensor_tensor(
                out=tbp[:], in0=aspv.unsqueeze(2).to_broadcast([NP, KT, H]),
                in1=heads_pre(1), op=OP.mult)
            nc.vector.tensor_tensor(out=ppre[:], in0=ppre[:], in1=tbp[:], op=OP.add)
            nc.vector.tensor_tensor(
                out=tbp[:], in0=adpv.unsqueeze(2).to_broadcast([NP, KT, H]),
                in1=heads_pre(2), op=OP.mult)
            nc.vector.tensor_tensor(out=ppre[:], in0=ppre[:], in1=tbp[:], op=OP.add)
            nc.vector.tensor_tensor(
                out=ppre[:], in0=ppre[:],
                in1=heads_pre(3), op=OP.add)
            epre = sb.tile([NP, KT, H], F32, name="epre")
            nc.scalar.activation(epre[:], ppre[:], ACT.Exp)
            dif = sb.tile([NP, KT], F32, name="dif")
            nc.vector.tensor_sub(dif[:], aspv, adpv)
            wpre = sb.tile([NP, KT], F32, name="wpre")
            nc.vector.tensor_tensor(out=wpre[:], in0=dif[:],
                                    in1=fsl("ea")[:NP, :], op=OP.mult)
            gxe = sb.tile([NP, KT, H], F32, name="gxe")
            nc.vector.tensor_tensor(
                out=gxe[:], in0=epre[:],
                in1=wpre[:].unsqueeze(2).to_broadcast([NP, KT, H]), op=OP.mult)
            gxb = sb.tile([NP, KT, H], BF16, name="gxb")
            nc.vector.tensor_tensor(
                out=gxb[:], in0=gxe[:],
                in1=rzb[:NP, :].unsqueeze(1).broadcast_to([NP, KT, H]),
                op=OP.mult)

            # ---------- ggx = B @ gxe ; x_new
            mh_s = sb.tile([NP, MH], F32, name="mhs")
            for m in range(MH):
                gp_ = psg.tile([NP, H], F32, tag="g")
                for k in range(KT):
                    nc.tensor.matmul(
                        out=gp_[:], lhsT=b_sb[:, k, m * NP:(m + 1) * NP],
                        rhs=gxb[:, k, :], start=(k == 0), stop=(k == KT - 1))
                nc.vector.reduce_sum(out=mh_s[:, m:m + 1], in_=gp_[:], axis=AX.X)

            xn = sb.tile([NP, MH, F], F32, name="xn")
            for m in range(MH):
                nc.vector.tensor_scalar(
                    out=xn[:, m, :],
                    in0=vcb[:NP, 0:2],
                    scalar1=mh_s[:, m:m + 1], scalar2=None, op0=OP.mult)
                nc.vector.tensor_tensor(
                    out=xn[:, m, :], in0=xn[:, m, :],
                    in1=vcb[:NP, 2:4], op=OP.add)
                nc.vector.tensor_tensor(
                    out=xn[:, m, :], in0=xn[:, m, :],
                    in1=xdl[:, m, 3, :], op=OP.add)
            nc.sync.dma_start(
                xnew[:].rearrange("(m p) f -> p m f", m=MH, p=NP), xn[:])

    nc.finalize()
    if split:
        _split_multi_waits(nc)
    return nc


_CACHE = {}


def _get_program(widths):
    key = (widths["ACH"], widths["BCH"], widths["SW"], widths["FWB"],
           widths["FWF"])
    if key not in _CACHE:
        _CACHE[key] = _build(widths)
    return _CACHE[key]


def kernel(**inputs) -> np.ndarray:
    from concourse.bass_utils import run_bass_kernel_spmd

    in_maps, widths, x = _prep(inputs)
    nc = _get_program(widths)
    res = run_bass_kernel_spmd(nc, in_maps, core_ids=list(range(C)))
    out = np.empty((1, T * N, F), np.float32)
    out[0, : (T - 1) * N] = x[N:]
    for c in range(C):
        out[0, (T - 1) * N + c * DSL:(T - 1) * N + (c + 1) * DSL] = \
            res.results[c]["xnew"]
    return out
